# revision 11
# baseline (speedup 1.0000x reference)
"""CIN (Compressed Interaction Network) kernel for Trainium2, 8 NeuronCores. v2.

Reference computation (per sample b, NFIELD=64, NEMB=64, NFILTER=128, 3 layers):
    xk_{l+1}[o, e] = relu( sum_{f,c} W_l[o, f*C+c] * x0[f, e] * xk_l[c, e] )
    pooled_l = sum_e xk_{l+1};  y = concat(pooled) @ Wa.T

Design (v2) — DVE-bound (~160us of tensor_tensor is the hard floor; every-
thing else is arranged to hide under it):
  - Data-parallel over batch: 32 samples/core, free axis J = 32*64 = 2048
    (b-major, e-minor), in NJ=4 free blocks of JB=512 (one PSUM bank).
  - K-permutation exploits operand reuse (weights are column-permuted on the
    host to match, which is free):
    * Layers 1-2 (K=64*128): K-tile t=8g+s has rows p -> (f, c) =
      (8g + p//16, 16s + p%16). The x0-side operand x0m8[p, g, e] =
      x0[8g+p//16] is host-replicated only x16 (vs x128 for the naive
      broadcast layout) and is reused by BOTH layers; the xk-side operand
      xkq[p, s, e] = xk[16s+p%16] is built on-chip per layer by 8 small
      partition-block copies + 3 partition-doubling DMAs (~1MB).
    * Layer 0 (symmetric, folded to 17 K-tiles): pairs are grouped by cyclic
      distance d: tile t<16 has rows (f, (f+d)%64), d = 2t+1+(p>=64); tile 16
      is the diagonal (bottom half zero-padded in W0'). Operands are
      x0stk[p, e] = x0[p%64] and host-gathered rotations
      x0r4[p, t, e] = x0[c(t,p)].
  - H tiles are materialized 4096-wide by DVE tensor_tensor with a stride-0
    broadcast middle dim on the x0-side operand ([128,1,512] ->
    to_broadcast([128,8,512])), which keeps the DVE 2x_1P perf mode
    (hardware-verified: 2290ns per 4096-elem bf16 TT).
  - Total DMA ~23MB/core (vs ~55MB for full x128 replication), all plain
    contiguous loads; LDWEIGHTS fully overlaps MATMUL so PE runs at the
    ~215ns/tile streaming rate and stays under the DVE.
  - Emission is wavefront-ordered (all cores' jj-blocks of layer l before
    layer l+1) so the DVE always has ready TT work while a stream is at a
    layer boundary (relu -> restack DMA chain).
"""

import sys

if "/opt/trn_rl_repo" not in sys.path:
    sys.path.insert(0, "/opt/trn_rl_repo")

import numpy as np
import ml_dtypes

B, F, E, O = 256, 64, 64, 128
NCORES = 8
BC = B // NCORES          # samples per core
J = BC * E                # free columns per core
JB = 512                  # free-block size (one PSUM bank)
NJ = J // JB              # 4 free blocks
KT0 = 17                  # layer-0 K-tiles (d-grouped symmetric fold)
NG = 8                    # layer 1-2 field groups (8 fields/group)

_BF16 = ml_dtypes.bfloat16
_STATE = {}


def _c0map():
    """Layer-0 gather map: c(t, p) for the d-rotation packing."""
    m = np.zeros((KT0, 128), np.int64)
    for t in range(KT0):
        for p in range(128):
            f = p % 64
            if t < 16:
                d = 2 * t + 1 + p // 64
                m[t, p] = (f + d) % 64
            else:
                m[t, p] = f
    return m


_C0MAP = _c0map()


def _build_nc():
    import concourse.tile as tile
    import concourse.mybir as mybir
    from concourse import bacc

    dt = mybir.dt
    nc = bacc.Bacc("TRN2", target_bir_lowering=False, debug=False)

    x0r4 = nc.dram_tensor(
        "x0r4", [NJ, 128, KT0 * JB], dt.bfloat16, kind="ExternalInput"
    )
    x0m8 = nc.dram_tensor(
        "x0m8", [NJ, 128, NG * JB], dt.bfloat16, kind="ExternalInput"
    )
    w0t = nc.dram_tensor("w0t", [128, KT0 * O], dt.bfloat16, kind="ExternalInput")
    w1t = nc.dram_tensor("w1t", [128, 64 * O], dt.bfloat16, kind="ExternalInput")
    w2t = nc.dram_tensor("w2t", [128, 64 * O], dt.bfloat16, kind="ExternalInput")
    wa = nc.dram_tensor("wa", [O, 3], dt.float32, kind="ExternalInput")
    y = nc.dram_tensor("y", [1, BC], dt.float32, kind="ExternalOutput")

    with tile.TileContext(nc) as tc:
        with (
            tc.tile_pool(name="wpool", bufs=1) as wpool,
            tc.tile_pool(name="stkpool", bufs=1) as stkpool,
            tc.tile_pool(name="r4pool", bufs=1) as r4pool,
            tc.tile_pool(name="hpool", bufs=4) as hpool,
            tc.tile_pool(name="xkqpool", bufs=4) as xkqpool,
            tc.tile_pool(name="xk2pool", bufs=2) as xk2pool,
            tc.tile_pool(name="dumpool", bufs=1) as dumpool,
            tc.tile_pool(name="psum", bufs=4, space="PSUM") as psum_pool,
            tc.tile_pool(name="psumy", bufs=1, space="PSUM") as psumy_pool,
        ):
            # --- persistent tiles -----------------------------------------
            w_sb = []
            for li, (wd, kt) in enumerate(zip((w0t, w1t, w2t), (KT0, 64, 64))):
                w_sb.append(
                    wpool.tile([128, kt, O], dt.bfloat16, tag=f"w{li}", name=f"w{li}")
                )
            wa_sb = wpool.tile([O, 3], dt.float32, tag="wa")
            pooled = [
                wpool.tile([O, BC], dt.float32, tag=f"pooled{l}", name=f"pooled{l}")
                for l in range(3)
            ]
            m8 = [
                stkpool.tile(
                    [128, NG, JB], dt.bfloat16, tag=f"m8_{jj}", name=f"m8_{jj}"
                )
                for jj in range(NJ)
            ]

            # --- upfront DMAs (ordered by first use; lead-in minimized) ---
            r4 = [None] * NJ

            def load_r4(jj, split):
                r4[jj] = r4pool.tile(
                    [128, KT0, JB], dt.bfloat16, tag=f"r4_{jj}", name=f"r4_{jj}"
                )
                nc.sync.dma_start(
                    r4[jj][:, 16, :], x0r4[jj][:, 16 * JB : KT0 * JB]
                )
                if split:
                    nc.sync.dma_start(
                        r4[jj][:, 0:4, :].rearrange("p t e -> p (t e)"),
                        x0r4[jj][:, 0 : 4 * JB],
                    )
                    nc.sync.dma_start(
                        r4[jj][:, 4:16, :].rearrange("p t e -> p (t e)"),
                        x0r4[jj][:, 4 * JB : 16 * JB],
                    )
                else:
                    nc.sync.dma_start(
                        r4[jj][:, 0:16, :].rearrange("p t e -> p (t e)"),
                        x0r4[jj][:, 0 : 16 * JB],
                    )

            load_r4(0, True)
            nc.sync.dma_start(w_sb[0][:].rearrange("p t o -> p (t o)"), w0t[:])
            load_r4(1, True)
            # first half of w1 right away: jj0's L1 matmuls need it ~30us in
            nc.sync.dma_start(
                w_sb[1][:, 0:32, :].rearrange("p t o -> p (t o)"),
                w1t[:, 0 : 32 * O],
            )
            nc.sync.dma_start(m8[0][:].rearrange("p g e -> p (g e)"), x0m8[0])
            load_r4(2, False)
            nc.sync.dma_start(
                w_sb[1][:, 32:64, :].rearrange("p t o -> p (t o)"),
                w1t[:, 32 * O : 64 * O],
            )
            nc.sync.dma_start(m8[1][:].rearrange("p g e -> p (g e)"), x0m8[1])
            load_r4(3, False)
            nc.sync.dma_start(wa_sb[:], wa[:])
            nc.sync.dma_start(
                w_sb[2][:].rearrange("p t o -> p (t o)"), w2t[:]
            )
            nc.sync.dma_start(m8[2][:].rearrange("p g e -> p (g e)"), x0m8[2])
            nc.sync.dma_start(m8[3][:].rearrange("p g e -> p (g e)"), x0m8[3])

            def restack(xk2, jj, tag):
                """xkq[p, s, e] = xk2[16s + p%16, e] via 8 copies + 3 doublings."""
                xkq = xkqpool.tile(
                    [128, NG, JB], dt.bfloat16, tag="xkq", name=f"xkq_{tag}_{jj}"
                )
                with tc.high_priority():
                    for s in range(NG):
                        nc.sync.dma_start(
                            xkq[0:16, s, :], xk2[16 * s : 16 * (s + 1), :]
                        )
                    nc.sync.dma_start(xkq[16:32, :, :], xkq[0:16, :, :])
                    nc.sync.dma_start(xkq[32:64, :, :], xkq[0:32, :, :])
                    nc.sync.dma_start(xkq[64:128, :, :], xkq[0:64, :, :])
                return xkq

            def epilogue(acc, jj, l, want_xk):
                xkq = None
                if want_xk:
                    xk2 = xk2pool.tile(
                        [128, JB], dt.bfloat16, tag="xk2", name=f"xk2_{jj}_{l}"
                    )
                    with tc.high_priority():
                        nc.scalar.activation(
                            xk2[:], acc[:], mybir.ActivationFunctionType.Relu
                        )
                    xkq = restack(xk2, jj, f"l{l}")
                dummy = dumpool.tile([128, JB], dt.bfloat16, tag="dummy")
                for b in range(8):
                    nc.scalar.activation(
                        dummy[:, E * b : E * (b + 1)],
                        acc[:, E * b : E * (b + 1)],
                        mybir.ActivationFunctionType.Relu,
                        accum_out=pooled[l][:, 8 * jj + b : 8 * jj + b + 1],
                    )
                return xkq

            # --- wave: layer 0 --------------------------------------------
            xkq_cur = [None] * NJ
            for jj in range(NJ):
                acc = psum_pool.tile(
                    [128, JB], dt.float32, tag="acc", name=f"acc{jj}_0"
                )
                hs = []
                for ci, (t0, nt) in enumerate(
                    ((0, 4), (4, 4), (8, 4), (12, 4), (16, 1))
                ):
                    h = hpool.tile(
                        [128, NG, JB], dt.bfloat16, tag="h", name=f"h0_{jj}_{ci}"
                    )
                    in1 = r4[jj][:, 16:17, :]
                    if nt > 1:
                        in1 = in1.to_broadcast([128, nt, JB])
                    nc.vector.tensor_tensor(
                        h[:, 0:nt, :],
                        in1,
                        r4[jj][:, t0 : t0 + nt, :],
                        op=mybir.AluOpType.mult,
                    )
                    for i in range(nt):
                        t = t0 + i
                        nc.tensor.matmul(
                            acc[:], w_sb[0][:, t, :], h[:, i, :],
                            start=(t == 0), stop=(t == KT0 - 1),
                        )
                    hs.append(h)
                xkq_cur[jj] = epilogue(acc, jj, 0, True)

            # --- waves: layers 1, 2 ---------------------------------------
            for l in (1, 2):
                xkq_next = [None] * NJ
                for jj in range(NJ):
                    acc = psum_pool.tile(
                        [128, JB], dt.float32, tag="acc", name=f"acc{jj}_{l}"
                    )
                    for g in range(NG):
                        h = hpool.tile(
                            [128, NG, JB], dt.bfloat16, tag="h",
                            name=f"h{l}_{jj}_{g}",
                        )
                        nc.vector.tensor_tensor(
                            h[:],
                            m8[jj][:, g : g + 1, :].to_broadcast([128, NG, JB]),
                            xkq_cur[jj][:],
                            op=mybir.AluOpType.mult,
                        )
                        for s in range(NG):
                            t = NG * g + s
                            nc.tensor.matmul(
                                acc[:], w_sb[l][:, t, :], h[:, s, :],
                                start=(t == 0), stop=(t == 63),
                            )
                    xkq_next[jj] = epilogue(acc, jj, l, l < 2)
                xkq_cur = xkq_next

            # --- head: y[b] = sum_l wa[:, l] . pooled[l][:, b] ------------
            yac = psumy_pool.tile([1, BC], dt.float32, tag="yac")
            for l in range(3):
                nc.tensor.matmul(
                    yac[:], wa_sb[:, l : l + 1], pooled[l][:],
                    start=(l == 0), stop=(l == 2),
                )
            y_sb = wpool.tile([1, BC], dt.float32, tag="ysb")
            nc.scalar.copy(y_sb[:], yac[:])
            nc.sync.dma_start(y[:], y_sb[:])

    nc.finalize()
    return nc


def _get_nc():
    if "nc" not in _STATE:
        _STATE["nc"] = _build_nc()
    return _STATE["nc"]


def _pack_w0(W0):
    """Fold symmetric pairs onto d-rotation tiles: [128, KT0*O] bf16."""
    w = np.asarray(W0, np.float32).reshape(O, F, F)
    wsym = w + w.transpose(0, 2, 1)  # folded (f, c) + (c, f)
    w0p = np.zeros((128, KT0, O), np.float32)
    for t in range(KT0):
        for p in range(128):
            f = p % 64
            if t < 16:
                d = 2 * t + 1 + p // 64
                c = (f + d) % 64
                if d == 32 and f >= 32:
                    continue
                w0p[p, t, :] = wsym[:, f, c]
            elif p < 64:
                w0p[p, t, :] = w[:, f, f]
    return w0p.reshape(128, KT0 * O)


def _pack_w(W):
    """Permute K to the (g, s) tile order: [128, 64*O] bf16."""
    w = np.asarray(W, np.float32).reshape(O, F, O)
    # K index of tile t row p: f = 8*(t//8) + p//16, c = 16*(t%8) + p%16
    t = np.arange(64)
    p = np.arange(128)
    f = 8 * (t[:, None] // 8) + p[None, :] // 16   # [t, p]
    c = 16 * (t[:, None] % 8) + p[None, :] % 16
    wp = w[:, f, c]                                 # [O, t, p]
    return np.ascontiguousarray(wp.transpose(2, 1, 0).reshape(128, 64 * O))


def _prep_in_maps(x, W0, W1, W2, Wa):
    x = np.asarray(x, dtype=np.float32)
    w0p = _pack_w0(W0).astype(_BF16)
    w1p = _pack_w(W1).astype(_BF16)
    w2p = _pack_w(W2).astype(_BF16)
    wa = np.ascontiguousarray(np.asarray(Wa, np.float32).reshape(3, O).T)

    p128 = np.arange(128)
    fmap = (8 * (np.arange(NG)[:, None]) + p128[None, :] // 16)  # [g, p]

    in_maps = []
    for cc in range(NCORES):
        xc = x[cc * BC : (cc + 1) * BC]                     # (BC, F, E)
        x0 = np.ascontiguousarray(xc.transpose(1, 0, 2).reshape(F, J)).astype(_BF16)

        r4_full = x0[_C0MAP]                                # [17, 128, J]
        x0r4 = np.ascontiguousarray(
            r4_full.transpose(1, 0, 2)                      # [128, 17, J]
            .reshape(128, KT0, NJ, JB)
            .transpose(2, 0, 1, 3)
            .reshape(NJ, 128, KT0 * JB)
        )
        m8_full = x0[fmap]                                  # [g, p, J]
        x0m8 = np.ascontiguousarray(
            m8_full.transpose(1, 0, 2)
            .reshape(128, NG, NJ, JB)
            .transpose(2, 0, 1, 3)
            .reshape(NJ, 128, NG * JB)
        )
        in_maps.append(
            {
                "x0r4": x0r4,
                "x0m8": x0m8,
                "w0t": w0p,
                "w1t": w1p,
                "w2t": w2p,
                "wa": wa,
            }
        )
    return in_maps


def _run(inputs, trace=False, **kwargs):
    from concourse.bass_utils import run_bass_kernel_spmd

    nc = _get_nc()
    in_maps = _prep_in_maps(**inputs)
    res = run_bass_kernel_spmd(
        nc, in_maps, core_ids=list(range(NCORES)), trace=trace, **kwargs
    )
    y = np.concatenate(
        [np.asarray(r["y"], np.float32).reshape(BC) for r in res.results]
    )
    return y, res


def kernel(**inputs) -> np.ndarray:
    y, _ = _run(inputs, trace=False)
    return y
